# revision 20
# baseline (speedup 1.0000x reference)
"""GPTQ 4-bit quantized linear (Autograd4bitQuantLinear) on 8 trn2 NeuronCores.

out[B,S,O] = x[B,S,K] @ dequant(qweight,qzeros,scales)[K,O] + bias[O]

Sharding: column-parallel over out_features (O=4096 -> 512 per core);
x replicated. Per core: dequantize the W shard to bf16 tiles in SBUF
(k on partitions; one 128-row group per k-tile), then matmul with W as
the stationary operand and x.T (host-pretransposed, bf16) as the moving
operand, accumulating out.T tiles [128o x 512m] in PSUM over the 32
k-tiles. Bias is added per-partition on PSUM eviction. Host gathers the
8 out.T shards, concatenates and transposes back.
"""

import contextlib
import sys

sys.path.insert(0, "/opt/trn_rl_repo")

import numpy as np
import ml_dtypes

import concourse.bass as bass
import concourse.mybir as mybir
import concourse.tile as tile
from concourse import bacc
from concourse.bass_utils import run_bass_kernel_spmd

# Full-problem dims (hardcoded per contract)
B, S = 4, 2048
K = 4096            # in_features
O = 4096            # out_features
GROUPSIZE = 128
N_CORES = 8
M = B * S           # 8192
O_SH = O // N_CORES # 512 per-core out shard

P = 128
F32 = mybir.dt.float32
I32 = mybir.dt.int32
BF16 = mybir.dt.bfloat16
Alu = mybir.AluOpType


def build_kernel(m=M, k=K, o_sh=O_SH, mw=512, reps=0, nsplit=1, brep=0):
    """Build the per-core SPMD program. m: rows of x; k: contraction;
    o_sh: out-feature shard; mw: m-chunk width (psum free dim).
    reps>0 wraps the whole body in a hardware loop (for timing)."""
    kt = k // P          # number of k tiles == number of groups
    ot = o_sh // P       # number of 128-row o tiles
    mch = m // mw        # number of m chunks

    nc = bacc.Bacc(None, target_bir_lowering=False, debug=False)

    xt = nc.declare_dram_parameter("xt", [k, m], BF16, isOutput=False)
    # qweight rows pre-expanded 8x on host: qwr[k, o] = qweight[k//8, o];
    # partition p of a k-tile holds nibble p%8 of its word
    qwr = nc.declare_dram_parameter("qwr", [k, o_sh], I32, isOutput=False)
    if brep:
        zp1 = nc.declare_dram_parameter("zp1r", [kt, P, o_sh], F32, isOutput=False)
        scale = nc.declare_dram_parameter("scaler", [kt, P, o_sh], F32, isOutput=False)
    else:
        zp1 = nc.declare_dram_parameter("zp1", [kt, o_sh], F32, isOutput=False)
        scale = nc.declare_dram_parameter("scale", [kt, o_sh], F32, isOutput=False)
    biasv = nc.declare_dram_parameter("biasv", [1, o_sh], F32, isOutput=False)
    shifts = nc.declare_dram_parameter("shifts", [P, 1], I32, isOutput=False)
    out = nc.declare_dram_parameter("out", [o_sh, m], F32, isOutput=True)

    with tile.TileContext(nc) as tc:
        with (
            tc.tile_pool(name="const", bufs=1) as const,
            tc.tile_pool(name="wpool", bufs=1) as wpool,
            tc.tile_pool(name="dq", bufs=3) as dq,
            tc.tile_pool(name="xpool", bufs=3) as xpool,
            tc.tile_pool(name="opool", bufs=3) as opool,
            tc.tile_pool(name="psum", bufs=min(8, (8 * 512) // mw), space="PSUM") as psum,
            contextlib.ExitStack() as body_ctx,
        ):
            if reps:
                body_ctx.enter_context(tc.For_i(0, reps, 1))
            shifts_sb = const.tile([P, 1], I32)
            nc.sync.dma_start(out=shifts_sb[:], in_=shifts[:])
            bias_sb = const.tile([P, ot], F32)
            # bias_sb[p, t] = biasv[0, t*128 + p]
            nc.sync.dma_start(
                out=bias_sb[:],
                in_=biasv[:].rearrange("1 (t p) -> p t", p=P),
            )

            # --- Phase A: dequantize W shard into SBUF bf16 tiles ---
            w_tiles = []
            for g in range(kt):
                w = wpool.tile([P, o_sh], BF16, tag=f"w{g}")
                w_tiles.append(w)

                qw_rep = dq.tile([P, o_sh], I32, tag="qwrep")
                nc.scalar.dma_start(
                    out=qw_rep[:], in_=qwr[g * P:(g + 1) * P, :]
                )
                zp1_b = dq.tile([P, o_sh], F32, tag="zp1b")
                scale_b = dq.tile([P, o_sh], F32, tag="scaleb")
                if brep:
                    nc.scalar.dma_start(out=zp1_b[:], in_=zp1[g])
                    nc.scalar.dma_start(out=scale_b[:], in_=scale[g])
                else:
                    nc.scalar.dma_start(
                        out=zp1_b[:], in_=zp1[g:g + 1, :].to_broadcast([P, o_sh])
                    )
                    nc.scalar.dma_start(
                        out=scale_b[:], in_=scale[g:g + 1, :].to_broadcast([P, o_sh])
                    )

                q_i32 = dq.tile([P, o_sh], I32, tag="qi32")
                nc.vector.tensor_tensor(
                    out=q_i32[:], in0=qw_rep[:],
                    in1=shifts_sb[:, 0:1].to_broadcast([P, o_sh]),
                    op=Alu.logical_shift_right,
                )
                nc.vector.tensor_scalar(
                    q_i32[:], q_i32[:], 0xF, None, Alu.bitwise_and
                )
                t_f32 = dq.tile([P, o_sh], F32, tag="tf32")
                # (q * 1.0) - (z+1); int32 -> f32 convert on output
                nc.vector.scalar_tensor_tensor(
                    t_f32[:], q_i32[:], 1.0, zp1_b[:], Alu.mult, Alu.subtract
                )
                nc.vector.tensor_tensor(
                    out=w[:], in0=t_f32[:], in1=scale_b[:], op=Alu.mult
                )

            # --- Phase B: matmul ---
            xt_r = xt[:].rearrange("(t p) m -> p t m", p=P)
            for mc in range(mch):
                xch = xpool.tile([P, kt, mw], BF16, tag="xch")
                nc.sync.dma_start(
                    out=xch[:], in_=xt_r[:, :, mc * mw:(mc + 1) * mw]
                )
                for t in range(ot):
                    ps = psum.tile([P, mw], F32, tag="ps")
                    nw = mw // nsplit
                    for g in range(kt):
                        for h in range(nsplit):
                            sl = slice(h * nw, (h + 1) * nw)
                            nc.tensor.matmul(
                                ps[:, sl],
                                lhsT=w_tiles[g][:, t * P:(t + 1) * P],
                                rhs=xch[:, g, sl],
                                start=(g == 0),
                                stop=(g == kt - 1),
                            )
                    osb = opool.tile([P, mw], F32, tag="osb")
                    nc.vector.tensor_scalar_add(osb[:], ps[:], bias_sb[:, t:t + 1])
                    nc.scalar.dma_start(
                        out=out[t * P:(t + 1) * P, mc * mw:(mc + 1) * mw],
                        in_=osb[:],
                    )

    nc.compile()
    return nc


def make_core_inputs(x, qweight, qzeros, scales, bias, m=M, k=K, brep=0):
    """Host-side prep: transpose x to [K, M] bf16, expand qweight rows 8x,
    unpack qzeros, shard along out features."""
    xt = np.ascontiguousarray(
        x.reshape(m, k).T.astype(ml_dtypes.bfloat16)
    )
    qwr_full = np.repeat(qweight, 8, axis=0)
    # unpack zeros: z[g, o] = (qzeros[g, o//8] >> 4*(o%8)) & 0xF
    qz = qzeros.astype(np.uint32)
    sh = (np.arange(8, dtype=np.uint32) * 4)[None, None, :]
    z = ((qz[:, :, None] >> sh) & np.uint32(0xF)).reshape(qzeros.shape[0], -1)
    zp1_full = (z.astype(np.float32) + 1.0)

    o_sh = qweight.shape[1] // N_CORES
    in_maps = []
    for c in range(N_CORES):
        sl = slice(c * o_sh, (c + 1) * o_sh)
        im = {
            "xt": xt,
            "qwr": np.ascontiguousarray(qwr_full[:, sl]),
            "biasv": np.ascontiguousarray(bias[sl]).reshape(1, o_sh),
            "shifts": (4 * (np.arange(P, dtype=np.int32) % 8)).reshape(P, 1),
        }
        if brep:
            im["zp1r"] = np.ascontiguousarray(
                np.broadcast_to(zp1_full[:, None, sl], (zp1_full.shape[0], P, o_sh)))
            im["scaler"] = np.ascontiguousarray(
                np.broadcast_to(scales[:, None, sl], (scales.shape[0], P, o_sh)))
        else:
            im["zp1"] = np.ascontiguousarray(zp1_full[:, sl])
            im["scale"] = np.ascontiguousarray(scales[:, sl])
        in_maps.append(im)
    return in_maps


_NC_CACHE = {}


def kernel(x, qweight, qzeros, scales, bias):
    if "nc" not in _NC_CACHE:
        _NC_CACHE["nc"] = build_kernel()
    nc = _NC_CACHE["nc"]
    in_maps = make_core_inputs(
        np.asarray(x), np.asarray(qweight), np.asarray(qzeros),
        np.asarray(scales), np.asarray(bias),
    )
    res = run_bass_kernel_spmd(nc, in_maps, list(range(N_CORES)))
    outT = np.concatenate([res.results[c]["out"] for c in range(N_CORES)], axis=0)
    return np.ascontiguousarray(outT.T).reshape(B, S, O).astype(np.float32)


# revision 21
# speedup vs baseline: 1.2505x; 1.2505x over previous
"""GPTQ 4-bit quantized linear (Autograd4bitQuantLinear) on 8 trn2 NeuronCores.

out[B,S,O] = x[B,S,K] @ dequant(qweight,qzeros,scales)[K,O] + bias[O]

Sharding: column-parallel over out_features (O=4096 -> 512 per core);
x replicated. Per core: dequantize the W shard to bf16 tiles in SBUF
(k on partitions; one 128-row group per k-tile), then matmul with W as
the stationary operand and x.T (host-pretransposed, bf16) as the moving
operand, accumulating out.T tiles [128o x 512m] in PSUM over the 32
k-tiles. Bias is added per-partition on PSUM eviction. Host gathers the
8 out.T shards, concatenates and transposes back.
"""

import contextlib
import sys

sys.path.insert(0, "/opt/trn_rl_repo")

import numpy as np
import ml_dtypes

import concourse.bass as bass
import concourse.mybir as mybir
import concourse.tile as tile
from concourse import bacc
from concourse.bass_utils import run_bass_kernel_spmd

# Full-problem dims (hardcoded per contract)
B, S = 4, 2048
K = 4096            # in_features
O = 4096            # out_features
GROUPSIZE = 128
N_CORES = 8
M = B * S           # 8192
O_SH = O // N_CORES # 512 per-core out shard

P = 128
F32 = mybir.dt.float32
I32 = mybir.dt.int32
BF16 = mybir.dt.bfloat16
Alu = mybir.AluOpType


def build_kernel(m=M, k=K, o_sh=O_SH, mw=512, reps=0, nsplit=1, brep=0, nodeq=0):
    """Build the per-core SPMD program. m: rows of x; k: contraction;
    o_sh: out-feature shard; mw: m-chunk width (psum free dim).
    reps>0 wraps the whole body in a hardware loop (for timing)."""
    kt = k // P          # number of k tiles == number of groups
    ot = o_sh // P       # number of 128-row o tiles
    mch = m // mw        # number of m chunks

    nc = bacc.Bacc(None, target_bir_lowering=False, debug=False)

    xt = nc.declare_dram_parameter("xt", [k, m], BF16, isOutput=False)
    # qweight rows pre-expanded 8x on host: qwr[k, o] = qweight[k//8, o];
    # partition p of a k-tile holds nibble p%8 of its word
    qwr = nc.declare_dram_parameter("qwr", [k, o_sh], I32, isOutput=False)
    if brep:
        zp1 = nc.declare_dram_parameter("zp1r", [kt, P, o_sh], F32, isOutput=False)
        scale = nc.declare_dram_parameter("scaler", [kt, P, o_sh], F32, isOutput=False)
    else:
        zp1 = nc.declare_dram_parameter("zp1", [kt, o_sh], F32, isOutput=False)
        scale = nc.declare_dram_parameter("scale", [kt, o_sh], F32, isOutput=False)
    biasv = nc.declare_dram_parameter("biasv", [1, o_sh], F32, isOutput=False)
    shifts = nc.declare_dram_parameter("shifts", [P, 1], I32, isOutput=False)
    out = nc.declare_dram_parameter("out", [o_sh, m], F32, isOutput=True)

    with tile.TileContext(nc) as tc:
        with (
            tc.tile_pool(name="const", bufs=1) as const,
            tc.tile_pool(name="wpool", bufs=1) as wpool,
            tc.tile_pool(name="dq", bufs=3) as dq,
            tc.tile_pool(name="xpool", bufs=3) as xpool,
            tc.tile_pool(name="opool", bufs=3) as opool,
            tc.tile_pool(name="psum", bufs=min(8, (8 * 512) // mw), space="PSUM") as psum,
            contextlib.ExitStack() as body_ctx,
        ):
            if reps:
                body_ctx.enter_context(tc.For_i(0, reps, 1))
            shifts_sb = const.tile([P, 1], I32)
            nc.sync.dma_start(out=shifts_sb[:], in_=shifts[:])
            bias_sb = const.tile([P, ot], F32)
            # bias_sb[p, t] = biasv[0, t*128 + p]
            nc.sync.dma_start(
                out=bias_sb[:],
                in_=biasv[:].rearrange("1 (t p) -> p t", p=P),
            )

            # --- Phase A: dequantize W shard into SBUF bf16 tiles ---
            w_tiles = []
            for g in range(kt):
                w = wpool.tile([P, o_sh], BF16, tag=f"w{g}")
                w_tiles.append(w)
                if nodeq:
                    nc.vector.memset(w[:], 0.25)
                    continue

                qw_rep = dq.tile([P, o_sh], I32, tag="qwrep")
                nc.scalar.dma_start(
                    out=qw_rep[:], in_=qwr[g * P:(g + 1) * P, :]
                )
                zp1_b = dq.tile([P, o_sh], F32, tag="zp1b")
                scale_b = dq.tile([P, o_sh], F32, tag="scaleb")
                if brep:
                    nc.scalar.dma_start(out=zp1_b[:], in_=zp1[g])
                    nc.scalar.dma_start(out=scale_b[:], in_=scale[g])
                else:
                    nc.scalar.dma_start(
                        out=zp1_b[:], in_=zp1[g:g + 1, :].to_broadcast([P, o_sh])
                    )
                    nc.scalar.dma_start(
                        out=scale_b[:], in_=scale[g:g + 1, :].to_broadcast([P, o_sh])
                    )

                q_i32 = dq.tile([P, o_sh], I32, tag="qi32")
                nc.vector.tensor_tensor(
                    out=q_i32[:], in0=qw_rep[:],
                    in1=shifts_sb[:, 0:1].to_broadcast([P, o_sh]),
                    op=Alu.logical_shift_right,
                )
                nc.vector.tensor_scalar(
                    q_i32[:], q_i32[:], 0xF, None, Alu.bitwise_and
                )
                t_f32 = dq.tile([P, o_sh], F32, tag="tf32")
                # (q * 1.0) - (z+1); int32 -> f32 convert on output
                nc.vector.scalar_tensor_tensor(
                    t_f32[:], q_i32[:], 1.0, zp1_b[:], Alu.mult, Alu.subtract
                )
                nc.vector.tensor_tensor(
                    out=w[:], in0=t_f32[:], in1=scale_b[:], op=Alu.mult
                )

            # --- Phase B: matmul ---
            xt_r = xt[:].rearrange("(t p) m -> p t m", p=P)
            for mc in range(mch):
                xch = xpool.tile([P, kt, mw], BF16, tag="xch")
                nc.sync.dma_start(
                    out=xch[:], in_=xt_r[:, :, mc * mw:(mc + 1) * mw]
                )
                for t in range(ot):
                    ps = psum.tile([P, mw], F32, tag="ps")
                    nw = mw // nsplit
                    for g in range(kt):
                        for h in range(nsplit):
                            sl = slice(h * nw, (h + 1) * nw)
                            nc.tensor.matmul(
                                ps[:, sl],
                                lhsT=w_tiles[g][:, t * P:(t + 1) * P],
                                rhs=xch[:, g, sl],
                                start=(g == 0),
                                stop=(g == kt - 1),
                            )
                    osb = opool.tile([P, mw], F32, tag="osb")
                    nc.vector.tensor_scalar_add(osb[:], ps[:], bias_sb[:, t:t + 1])
                    nc.scalar.dma_start(
                        out=out[t * P:(t + 1) * P, mc * mw:(mc + 1) * mw],
                        in_=osb[:],
                    )

    nc.compile()
    return nc


def make_core_inputs(x, qweight, qzeros, scales, bias, m=M, k=K, brep=0):
    """Host-side prep: transpose x to [K, M] bf16, expand qweight rows 8x,
    unpack qzeros, shard along out features."""
    xt = np.ascontiguousarray(
        x.reshape(m, k).T.astype(ml_dtypes.bfloat16)
    )
    qwr_full = np.repeat(qweight, 8, axis=0)
    # unpack zeros: z[g, o] = (qzeros[g, o//8] >> 4*(o%8)) & 0xF
    qz = qzeros.astype(np.uint32)
    sh = (np.arange(8, dtype=np.uint32) * 4)[None, None, :]
    z = ((qz[:, :, None] >> sh) & np.uint32(0xF)).reshape(qzeros.shape[0], -1)
    zp1_full = (z.astype(np.float32) + 1.0)

    o_sh = qweight.shape[1] // N_CORES
    in_maps = []
    for c in range(N_CORES):
        sl = slice(c * o_sh, (c + 1) * o_sh)
        im = {
            "xt": xt,
            "qwr": np.ascontiguousarray(qwr_full[:, sl]),
            "biasv": np.ascontiguousarray(bias[sl]).reshape(1, o_sh),
            "shifts": (4 * (np.arange(P, dtype=np.int32) % 8)).reshape(P, 1),
        }
        if brep:
            im["zp1r"] = np.ascontiguousarray(
                np.broadcast_to(zp1_full[:, None, sl], (zp1_full.shape[0], P, o_sh)))
            im["scaler"] = np.ascontiguousarray(
                np.broadcast_to(scales[:, None, sl], (scales.shape[0], P, o_sh)))
        else:
            im["zp1"] = np.ascontiguousarray(zp1_full[:, sl])
            im["scale"] = np.ascontiguousarray(scales[:, sl])
        in_maps.append(im)
    return in_maps


_NC_CACHE = {}


def kernel(x, qweight, qzeros, scales, bias):
    if "nc" not in _NC_CACHE:
        _NC_CACHE["nc"] = build_kernel()
    nc = _NC_CACHE["nc"]
    in_maps = make_core_inputs(
        np.asarray(x), np.asarray(qweight), np.asarray(qzeros),
        np.asarray(scales), np.asarray(bias),
    )
    res = run_bass_kernel_spmd(nc, in_maps, list(range(N_CORES)))
    outT = np.concatenate([res.results[c]["out"] for c in range(N_CORES)], axis=0)
    return np.ascontiguousarray(outT.T).reshape(B, S, O).astype(np.float32)


# revision 22
# speedup vs baseline: 1.2588x; 1.0067x over previous
"""GPTQ 4-bit quantized linear (Autograd4bitQuantLinear) on 8 trn2 NeuronCores.

out[B,S,O] = x[B,S,K] @ dequant(qweight,qzeros,scales)[K,O] + bias[O]

Sharding: column-parallel over out_features (O=4096 -> 512 per core);
x replicated. Per core: dequantize the W shard to bf16 tiles in SBUF
(k on partitions; one 128-row group per k-tile), then matmul with W as
the stationary operand and x.T (host-pretransposed, bf16) as the moving
operand, accumulating out.T tiles [128o x 512m] in PSUM over the 32
k-tiles. Bias is added per-partition on PSUM eviction. Host gathers the
8 out.T shards, concatenates and transposes back.
"""

import contextlib
import sys

sys.path.insert(0, "/opt/trn_rl_repo")

import numpy as np
import ml_dtypes

import concourse.bass as bass
import concourse.mybir as mybir
import concourse.tile as tile
from concourse import bacc
from concourse.bass_utils import run_bass_kernel_spmd

# Full-problem dims (hardcoded per contract)
B, S = 4, 2048
K = 4096            # in_features
O = 4096            # out_features
GROUPSIZE = 128
N_CORES = 8
M = B * S           # 8192
O_SH = O // N_CORES # 512 per-core out shard

P = 128
F32 = mybir.dt.float32
I32 = mybir.dt.int32
BF16 = mybir.dt.bfloat16
Alu = mybir.AluOpType


def build_kernel(m=M, k=K, o_sh=O_SH, mw=512, reps=0, nsplit=1, brep=0, nodeq=0,
                 preshift=0, noxdma=0):
    """Build the per-core SPMD program. m: rows of x; k: contraction;
    o_sh: out-feature shard; mw: m-chunk width (psum free dim).
    reps>0 wraps the whole body in a hardware loop (for timing)."""
    kt = k // P          # number of k tiles == number of groups
    ot = o_sh // P       # number of 128-row o tiles
    mch = m // mw        # number of m chunks

    nc = bacc.Bacc(None, target_bir_lowering=False, debug=False)

    xt = nc.declare_dram_parameter("xt", [k, m], BF16, isOutput=False)
    # qweight rows pre-expanded 8x on host: qwr[k, o] = qweight[k//8, o];
    # partition p of a k-tile holds nibble p%8 of its word
    qwr = nc.declare_dram_parameter("qwr", [k, o_sh], I32, isOutput=False)
    if brep:
        zp1 = nc.declare_dram_parameter("zp1r", [kt, P, o_sh], F32, isOutput=False)
        scale = nc.declare_dram_parameter("scaler", [kt, P, o_sh], F32, isOutput=False)
    else:
        zp1 = nc.declare_dram_parameter("zp1", [kt, o_sh], F32, isOutput=False)
        scale = nc.declare_dram_parameter("scale", [kt, o_sh], F32, isOutput=False)
    biasv = nc.declare_dram_parameter("biasv", [1, o_sh], F32, isOutput=False)
    shifts = nc.declare_dram_parameter("shifts", [P, 1], I32, isOutput=False)
    out = nc.declare_dram_parameter("out", [o_sh, m], F32, isOutput=True)

    with tile.TileContext(nc) as tc:
        with (
            tc.tile_pool(name="const", bufs=1) as const,
            tc.tile_pool(name="wpool", bufs=1) as wpool,
            tc.tile_pool(name="dq", bufs=3) as dq,
            tc.tile_pool(name="xpool", bufs=3) as xpool,
            tc.tile_pool(name="opool", bufs=3) as opool,
            tc.tile_pool(name="psum", bufs=min(8, (8 * 512) // mw), space="PSUM") as psum,
            contextlib.ExitStack() as body_ctx,
        ):
            if reps:
                body_ctx.enter_context(tc.For_i(0, reps, 1))
            shifts_sb = const.tile([P, 1], I32)
            nc.sync.dma_start(out=shifts_sb[:], in_=shifts[:])
            bias_sb = const.tile([P, ot], F32)
            # bias_sb[p, t] = biasv[0, t*128 + p]
            nc.sync.dma_start(
                out=bias_sb[:],
                in_=biasv[:].rearrange("1 (t p) -> p t", p=P),
            )

            # --- Phase A: dequantize W shard into SBUF bf16 tiles ---
            w_tiles = []
            for g in range(kt):
                w = wpool.tile([P, o_sh], BF16, tag=f"w{g}")
                w_tiles.append(w)
                if nodeq:
                    nc.vector.memset(w[:], 0.25)
                    continue

                qw_rep = dq.tile([P, o_sh], I32, tag="qwrep")
                nc.scalar.dma_start(
                    out=qw_rep[:], in_=qwr[g * P:(g + 1) * P, :]
                )
                zp1_b = dq.tile([P, o_sh], F32, tag="zp1b")
                scale_b = dq.tile([P, o_sh], F32, tag="scaleb")
                if brep:
                    nc.scalar.dma_start(out=zp1_b[:], in_=zp1[g])
                    nc.scalar.dma_start(out=scale_b[:], in_=scale[g])
                else:
                    nc.scalar.dma_start(
                        out=zp1_b[:], in_=zp1[g:g + 1, :].to_broadcast([P, o_sh])
                    )
                    nc.scalar.dma_start(
                        out=scale_b[:], in_=scale[g:g + 1, :].to_broadcast([P, o_sh])
                    )

                q_i32 = dq.tile([P, o_sh], I32, tag="qi32")
                if preshift:
                    nc.vector.tensor_scalar(
                        q_i32[:], qw_rep[:], 0xF, None, Alu.bitwise_and
                    )
                else:
                    nc.vector.tensor_tensor(
                        out=q_i32[:], in0=qw_rep[:],
                        in1=shifts_sb[:, 0:1].to_broadcast([P, o_sh]),
                        op=Alu.logical_shift_right,
                    )
                    nc.vector.tensor_scalar(
                        q_i32[:], q_i32[:], 0xF, None, Alu.bitwise_and
                    )
                t_f32 = dq.tile([P, o_sh], F32, tag="tf32")
                # (q * 1.0) - (z+1); int32 -> f32 convert on output
                nc.vector.scalar_tensor_tensor(
                    t_f32[:], q_i32[:], 1.0, zp1_b[:], Alu.mult, Alu.subtract
                )
                nc.vector.tensor_tensor(
                    out=w[:], in0=t_f32[:], in1=scale_b[:], op=Alu.mult
                )

            # --- Phase B: matmul ---
            xt_r = xt[:].rearrange("(t p) m -> p t m", p=P)
            for mc in range(mch):
                xch = xpool.tile([P, kt, mw], BF16, tag="xch")
                xsrc = 0 if noxdma else mc
                nc.sync.dma_start(
                    out=xch[:], in_=xt_r[:, :, xsrc * mw:(xsrc + 1) * mw]
                )
                for t in range(ot):
                    ps = psum.tile([P, mw], F32, tag="ps")
                    nw = mw // nsplit
                    for g in range(kt):
                        for h in range(nsplit):
                            sl = slice(h * nw, (h + 1) * nw)
                            nc.tensor.matmul(
                                ps[:, sl],
                                lhsT=w_tiles[g][:, t * P:(t + 1) * P],
                                rhs=xch[:, g, sl],
                                start=(g == 0),
                                stop=(g == kt - 1),
                            )
                    osb = opool.tile([P, mw], F32, tag="osb")
                    nc.vector.tensor_scalar_add(osb[:], ps[:], bias_sb[:, t:t + 1])
                    nc.scalar.dma_start(
                        out=out[t * P:(t + 1) * P, mc * mw:(mc + 1) * mw],
                        in_=osb[:],
                    )

    nc.compile()
    return nc


def make_core_inputs(x, qweight, qzeros, scales, bias, m=M, k=K, brep=0, preshift=0):
    """Host-side prep: transpose x to [K, M] bf16, expand qweight rows 8x,
    unpack qzeros, shard along out features."""
    xt = np.ascontiguousarray(
        x.reshape(m, k).T.astype(ml_dtypes.bfloat16)
    )
    if preshift:
        s = (4 * (np.arange(qweight.shape[0] * 8, dtype=np.uint32) % 8))[:, None]
        qwr_full = (np.repeat(qweight.astype(np.uint32), 8, axis=0) >> s).view(np.int32)
    else:
        qwr_full = np.repeat(qweight, 8, axis=0)
    # unpack zeros: z[g, o] = (qzeros[g, o//8] >> 4*(o%8)) & 0xF
    qz = qzeros.astype(np.uint32)
    sh = (np.arange(8, dtype=np.uint32) * 4)[None, None, :]
    z = ((qz[:, :, None] >> sh) & np.uint32(0xF)).reshape(qzeros.shape[0], -1)
    zp1_full = (z.astype(np.float32) + 1.0)

    o_sh = qweight.shape[1] // N_CORES
    in_maps = []
    for c in range(N_CORES):
        sl = slice(c * o_sh, (c + 1) * o_sh)
        im = {
            "xt": xt,
            "qwr": np.ascontiguousarray(qwr_full[:, sl]),
            "biasv": np.ascontiguousarray(bias[sl]).reshape(1, o_sh),
            "shifts": (4 * (np.arange(P, dtype=np.int32) % 8)).reshape(P, 1),
        }
        if brep:
            im["zp1r"] = np.ascontiguousarray(
                np.broadcast_to(zp1_full[:, None, sl], (zp1_full.shape[0], P, o_sh)))
            im["scaler"] = np.ascontiguousarray(
                np.broadcast_to(scales[:, None, sl], (scales.shape[0], P, o_sh)))
        else:
            im["zp1"] = np.ascontiguousarray(zp1_full[:, sl])
            im["scale"] = np.ascontiguousarray(scales[:, sl])
        in_maps.append(im)
    return in_maps


_NC_CACHE = {}


def kernel(x, qweight, qzeros, scales, bias):
    if "nc" not in _NC_CACHE:
        _NC_CACHE["nc"] = build_kernel()
    nc = _NC_CACHE["nc"]
    in_maps = make_core_inputs(
        np.asarray(x), np.asarray(qweight), np.asarray(qzeros),
        np.asarray(scales), np.asarray(bias),
    )
    res = run_bass_kernel_spmd(nc, in_maps, list(range(N_CORES)))
    outT = np.concatenate([res.results[c]["out"] for c in range(N_CORES)], axis=0)
    return np.ascontiguousarray(outT.T).reshape(B, S, O).astype(np.float32)
